# revision 17
# baseline (speedup 1.0000x reference)
"""v3 reference re-run: original baseline structure with bf16 h in DRAM."""

import sys

if "/opt/trn_rl_repo" not in sys.path:
    sys.path.insert(0, "/opt/trn_rl_repo")

import numpy as np

B, C, A, NT, L, T = 32, 16, 4, 2048, 16, 2063
MH, MQ = 2, 1032  # padded T = 2064 = MH * MQ
P = 128  # partitions = MH * A * L
NCORES = 8
BS = B // NCORES  # batches per core
NBLK = ((0, 512), (512, 512), (1024, 8))  # mq -> psum bank blocks
CBLK = 4  # c's per h DMA (2.1 MB transfers)
HBUFS = 5
PBUFS = 8

TRACE = False
LAST = {}

_CACHE = {}


def _build_nc():
    import concourse.bacc as bacc
    import concourse.mybir as mybir
    import concourse.tile as tile

    f32 = mybir.dt.float32
    bf16 = mybir.dt.bfloat16

    nc = bacc.Bacc("TRN2", target_bir_lowering=False, debug=False)
    hh = nc.dram_tensor("hh", [BS, 4, P, 4, MQ], bf16, kind="ExternalInput")
    vv = nc.dram_tensor("vv", [BS, P, MQ], bf16, kind="ExternalInput")
    ww = nc.dram_tensor("ww", [P, C * 32], bf16, kind="ExternalInput")
    out = nc.dram_tensor("out", [BS, 2 * C, MQ], f32, kind="ExternalOutput")

    from concourse.tile import add_dep_helper

    with tile.TileContext(nc) as tc:
        with (
            tc.tile_pool(name="wpool", bufs=1) as wpool,
            tc.tile_pool(name="vpool", bufs=BS) as vpool,
            tc.tile_pool(name="hpool", bufs=HBUFS) as hpool,
            tc.tile_pool(name="ppool", bufs=PBUFS) as ppool,
            tc.tile_pool(name="ypool", bufs=2) as ypool,
            tc.tile_pool(name="pspool", bufs=6, space="PSUM") as pspool,
        ):
            wb = wpool.tile([P, C * 32], bf16)
            nc.scalar.dma_start(out=wb[:], in_=ww[:])
            wsc = wpool.tile([P, 32], bf16, tag="wsc")
            nc.vector.memset(wsc[:], 0)
            xsc = wpool.tile([P, 512], bf16, tag="xsc")
            nc.vector.memset(xsc[:], 0)
            pssc = pspool.tile([32, 512], f32, tag="pssc", bufs=1)
            warm_prev = None
            for i in range(18):
                wmm = nc.tensor.matmul(
                    out=pssc[:], lhsT=wsc[:], rhs=xsc[:], start=True, stop=True
                )
                if warm_prev is not None:
                    add_dep_helper(wmm.ins, warm_prev, sync=False,
                                   reason="warmup chain")
                warm_prev = wmm.ins
            vts = []
            for b in range(BS):
                vt = vpool.tile([P, MQ], bf16, tag="v", name=f"v{b}")
                nc.sync.dma_start(out=vt[:], in_=vv[b])
                vts.append(vt)

            def cblocks(b):
                if b == BS - 1:
                    return [4, 4, 4, 2, 1, 1]
                return [CBLK] * (C // CBLK)

            for b in range(BS):
                psums = [
                    pspool.tile([2 * C, n], f32, tag="psum", name=f"ps{b}_{i}")
                    for i, (_, n) in enumerate(NBLK)
                ]

                def mms(pt, c, lo, hi):
                    for blk, (off, n) in enumerate(NBLK):
                        if off >= hi or off + n <= lo:
                            continue
                        nc.tensor.matmul(
                            out=psums[blk][:, :],
                            lhsT=wb[:, c * 32 : (c + 1) * 32],
                            rhs=pt[:, off : off + n],
                            start=(c == 0),
                            stop=(c == C - 1),
                        )

                for q in range(4):
                    ht = hpool.tile([P, CBLK, MQ], bf16, tag="ht")
                    nc.gpsimd.dma_start(out=ht[:], in_=hh[b, q])
                    pt = ppool.tile([P, 4, MQ], bf16)
                    nc.vector.tensor_mul(
                        out=pt[:],
                        in0=ht[:],
                        in1=vts[b][:].unsqueeze(1).broadcast_to([P, 4, MQ]),
                    )
                    for cc in range(4):
                        mms(pt[:, cc, :], 4 * q + cc, 0, MQ)
                if b < BS - 1:
                    yt = ypool.tile([2 * C, MQ], f32)
                    for blk, (off, n) in enumerate(NBLK):
                        eng = nc.vector if blk == 1 else nc.scalar
                        if eng is nc.vector:
                            eng.tensor_copy(
                                out=yt[:, off : off + n], in_=psums[blk][:, :]
                            )
                        else:
                            eng.copy(out=yt[:, off : off + n], in_=psums[blk][:, :])
                    nc.scalar.dma_start(out=out[b], in_=yt[:])
                else:
                    y2 = ypool.tile([2 * C, 8], f32, tag="y2")
                    nc.scalar.copy(out=y2[:], in_=psums[2][:, :])
                    nc.sync.dma_start(out=out[b, :, 1024:MQ], in_=y2[:])
                    y0 = ypool.tile([2 * C, 512], f32, tag="y0")
                    nc.scalar.copy(out=y0[:], in_=psums[0][:, :])
                    nc.sync.dma_start(out=out[b, :, 0:512], in_=y0[:])
                    y1 = ypool.tile([2 * C, 512], f32, tag="y1")
                    nc.vector.tensor_copy(out=y1[:], in_=psums[1][:, :])
                    nc.scalar.dma_start(out=out[b, :, 512:1024], in_=y1[:])

    nc.compile()
    return nc


def _get_nc():
    if "nc" not in _CACHE:
        _CACHE["nc"] = _build_nc()
    return _CACHE["nc"]


def _make_ww():
    import ml_dtypes
    ww = np.zeros((P, C * 32), np.float32)
    for c in range(C):
        for mh in range(MH):
            ww[mh * 64 : (mh + 1) * 64, c * 32 + 2 * c + mh] = 1.0
    return ww.astype(ml_dtypes.bfloat16)


def _prep_inputs(x, h_time, g):
    x = np.asarray(x, dtype=np.float32)
    h = np.asarray(h_time, dtype=np.float32)
    g = np.asarray(g)

    xsq = x.reshape(B, A, NT)
    xp = np.zeros((B, A, NT + 1), np.float32)
    xp[:, :, :NT] = xsq
    gi = np.clip(g.astype(np.int64), 0, NT)
    xg = xp[:, :, gi]  # [B, A, T, L]

    xgp = np.zeros((B, A, MH * MQ, L), np.float32)
    xgp[:, :, :T] = xg
    import ml_dtypes
    vv = xgp.reshape(B, A, MH, MQ, L).transpose(0, 2, 1, 4, 3).reshape(B, P, MQ)
    vv = np.ascontiguousarray(vv).astype(ml_dtypes.bfloat16)

    hsq = h.reshape(B, C, A, T, L)
    hp = np.zeros((B, C, A, MH * MQ, L), np.float32)
    hp[:, :, :, :T] = hsq
    hh = (
        hp.reshape(B, C, A, MH, MQ, L)
        .transpose(0, 3, 2, 5, 1, 4)
        .reshape(B, P, C, MQ)
    )
    hh = hh.reshape(B, P, 4, 4, MQ).transpose(0, 2, 1, 3, 4)
    hh = np.ascontiguousarray(hh).astype(ml_dtypes.bfloat16)
    return hh, vv, _make_ww()


def _postprocess(res_list):
    y = np.concatenate([np.asarray(r["out"]) for r in res_list], axis=0)
    y = y.reshape(B, C, MH, MQ).reshape(B, C, MH * MQ)[:, :, :T]
    return np.ascontiguousarray(y.reshape(B, 1, C, T).astype(np.float32))


def kernel(x, h_time, g):
    from concourse.bass_utils import run_bass_kernel_spmd

    hh, vv, ww = _prep_inputs(x, h_time, g)
    in_maps = []
    for i in range(NCORES):
        sl = slice(i * BS, (i + 1) * BS)
        in_maps.append({"hh": hh[sl], "vv": vv[sl], "ww": ww})

    nc = _get_nc()
    kw = {}
    if TRACE and LAST.get("trace_cores"):
        kw["trace_cores"] = LAST["trace_cores"]
    res = run_bass_kernel_spmd(
        nc, in_maps, core_ids=list(range(NCORES)), trace=TRACE, **kw
    )
    LAST["exec_time_ns"] = res.exec_time_ns
    LAST["result"] = res
    return _postprocess(res.results)
